# revision 1
# baseline (speedup 1.0000x reference)
"""Bass/Trainium2 kernel for nn_Attention (ragged masked-softmax attention).

Math (per batch b with valid length L):
    c_b      = W_h @ hidden[:, b] + b_attn                  # [2H], W_h = W_attn[:, :H]
    e[s, :]  = tanh(W_e @ x_s + c_b)                        # W_e = W_attn[:, H:]
    score[s] = w_v . e[s, :] + b_v            (s < L)
    energy   = softmax(score[:L]);  context = energy @ X[:L]

Device strategy: the ragged work is split into fixed 256-position chunks
("units", 72 total for the graded lengths), distributed evenly over 8 cores
(one identical static SPMD program; per-core behavior differs only through
data).  Each unit produces flash-softmax partials (m, Z, ctx) which the host
merges exactly.  Matmul operands are fp16 (full-rate on the PE, fp32 PSUM
accumulation); softmax is fp32.
"""

import numpy as np

import concourse.bass as bass
import concourse.mybir as mybir
import concourse.tile as tile
from concourse import bacc
from concourse.bass_utils import run_bass_kernel_spmd

B, S, H = 16, 2048, 1024
H2 = 2 * H            # 2048 output features / encoder dim
CHUNK = 256           # sequence positions per work unit
N_CORES = 8
FB = H2 // 128        # 16 f-blocks of the contraction dim (encoder features)
OB = H2 // 128        # 16 o-blocks of the output features
HB = H // 128         # 8 h-blocks of the hidden contraction
NEG = -30000.0        # masked-score offset (exp underflows to exactly 0)

F16 = mybir.dt.float16
F32 = mybir.dt.float32


def build_program(nchunk: int, nhb: int = HB + 1):
    nc = bacc.Bacc()

    xt_ext = nc.declare_dram_parameter("xt", [nchunk, 128, FB, CHUNK], F16, isOutput=False)
    xn_ext = nc.declare_dram_parameter("xn", [nchunk, 128, CHUNK // 128, H2], F16, isOutput=False)
    mask_ext = nc.declare_dram_parameter("mask", [nchunk, CHUNK], F32, isOutput=False)
    hu_ext = nc.declare_dram_parameter("hu", [128, nhb, nchunk], F16, isOutput=False)
    # weights are staged o-block-major so compute can start after ~1MB of DMA
    wet_ext = nc.declare_dram_parameter("wet", [OB, 128, FB, 128], F16, isOutput=False)
    wht_ext = nc.declare_dram_parameter("wht", [OB, 128, nhb, 128], F16, isOutput=False)
    wv_ext = nc.declare_dram_parameter("wv", [128, OB], F16, isOutput=False)
    ctx_out = nc.declare_dram_parameter("out_ctx", [nchunk, H2], F32, isOutput=True)
    mz_out = nc.declare_dram_parameter("out_mz", [nchunk, 2], F32, isOutput=True)

    SB = CHUNK // 128   # s-blocks per unit for the context matmul
    DQ = H2 // 512      # 512-wide output quarters for the context matmul

    from contextlib import ExitStack
    with tile.TileContext(nc) as tc, ExitStack() as stk:
        singles = stk.enter_context(tc.tile_pool(name="singles", bufs=1))
        xtp = stk.enter_context(tc.tile_pool(name="xtp", bufs=2))
        xnp = stk.enter_context(tc.tile_pool(name="xnp", bufs=3))
        tp = stk.enter_context(tc.tile_pool(name="tp", bufs=2))
        smalls = stk.enter_context(tc.tile_pool(name="smalls", bufs=3))
        eps = stk.enter_context(tc.tile_pool(name="eps", bufs=3, space="PSUM"))
        sps = stk.enter_context(tc.tile_pool(name="sps", bufs=2, space="PSUM"))
        cps = stk.enter_context(tc.tile_pool(name="cps", bufs=2, space="PSUM"))

        # resident weights as one tile per o-block (fine-grained DMA deps so
        # the PE can start as soon as the first o-block's weights land)
        wet_sb = []
        wht_sb = []
        hu_sb = singles.tile([128, nhb, nchunk], F16)
        wv_sb = singles.tile([128, OB], F16)
        mask_sb = singles.tile([1, nchunk, CHUNK], F32)
        xt0_sb = xtp.tile([128, FB, CHUNK], F16, tag="xt")
        for ob in range(OB):
            if ob == 0:
                # the very first PE work is C(0) = wht0 x hu: land those first
                nc.sync.dma_start(out=hu_sb[:], in_=hu_ext[:])
            w2 = singles.tile([128, nhb, 128], F16, tag=f"wht{ob}")
            nc.sync.dma_start(out=w2[:], in_=wht_ext[ob])
            w1 = singles.tile([128, FB, 128], F16, tag=f"wet{ob}")
            nc.sync.dma_start(out=w1[:], in_=wet_ext[ob])
            wet_sb.append(w1)
            wht_sb.append(w2)
            if ob == 0:
                nc.sync.dma_start(out=wv_sb[:], in_=wv_ext[:])
                nc.sync.dma_start(out=mask_sb[0:1, :, :], in_=mask_ext[:])
                nc.sync.dma_start(out=xt0_sb[:], in_=xt_ext[0])
        mz_all = singles.tile([1, nchunk, 2], F32)
        ident_sb = singles.tile([1, 1], F16)
        nc.vector.memset(ident_sb[:], 1.0)

        # per-unit bias columns: c[o, i] = sum_h W_h[o, h] hu[h, i] (+ b_attn
        # row).  Emitted lazily inside unit 0's ob loop so each C(ob) group
        # sits right before the e-group that unblocks tanh(ob).
        c_sb = [None] * OB

        def emit_c(ob):
            c_ps = cps.tile([128, nchunk], F32, tag="cps")
            for jh in range(nhb):
                nc.tensor.matmul(
                    c_ps[:],
                    lhsT=wht_sb[ob][:, jh, :],
                    rhs=hu_sb[:, jh, :],
                    start=(jh == 0), stop=(jh == nhb - 1),
                )
            c1 = singles.tile([128, nchunk], F32, tag=f"c{ob}")
            nc.vector.tensor_copy(out=c1[:], in_=c_ps[:])
            c_sb[ob] = c1

        def emit_xn_dma(p):
            i, xn_sb = p[0], p[2]
            nc.sync.dma_start(out=xn_sb[:], in_=xn_ext[i])

        def emit_ctx(p):
            i, pt_sb, xn_sb = p[0], p[1], p[2]
            ctx_sb = smalls.tile([1, H2], F32, tag="ctx")
            for dq in range(DQ):
                ctx_ps = cps.tile([1, 512], F32, tag="cps")
                for sb in range(SB):
                    nc.tensor.matmul(
                        ctx_ps[:],
                        lhsT=pt_sb[:, sb:sb + 1],
                        rhs=xn_sb[:, sb, dq * 512:(dq + 1) * 512],
                        start=(sb == 0), stop=(sb == SB - 1),
                    )
                if dq % 2 == 0:
                    nc.vector.tensor_copy(out=ctx_sb[0:1, dq * 512:(dq + 1) * 512], in_=ctx_ps[:])
                else:
                    nc.scalar.copy(out=ctx_sb[0:1, dq * 512:(dq + 1) * 512], in_=ctx_ps[:])
            nc.sync.dma_start(out=ctx_out[i], in_=ctx_sb[0:1, :])

        def emit_egroup(i, xt_sb, t_sb, ob):
            if c_sb[ob] is None:
                emit_c(ob)
            e_ps = eps.tile([128, CHUNK], F32, tag="e")
            for fb in range(FB):
                nc.tensor.matmul(
                    e_ps[:],
                    lhsT=wet_sb[ob][:, fb, :],
                    rhs=xt_sb[:, fb, :],
                    start=(fb == 0), stop=(fb == FB - 1),
                )
            nc.scalar.activation(
                out=t_sb[:, ob, :], in_=e_ps[:],
                func=mybir.ActivationFunctionType.Tanh,
                bias=c_sb[ob][:, i:i + 1], scale=1.0,
            )

        def emit_scores(i, t_sb):
            # scores[s] = sum_o w_v[o] t[o, s] -> 4 partial rows (PE column
            # groups run concurrently; tile_position derives from the slices)
            s_ps = sps.tile([128, CHUNK], F32, tag="s", bufs=1)
            for r in range(OB // 4):
                for j in range(4):
                    ob = r * 4 + j
                    nc.tensor.matmul(
                        s_ps[32 * j:32 * j + 1, :],
                        lhsT=wv_sb[:, ob:ob + 1],
                        rhs=t_sb[:, ob, :],
                        start=(r == 0), stop=(r == OB // 4 - 1),
                        tile_position=(0, 32 * j),
                    )
            return s_ps

        def emit_softmax(i, s_ps):
            # masked softmax partials: fold the 4 partial rows + mask
            # (DVE may read at most one PSUM operand per op -> serial chain)
            acc_sb = []
            for j in range(4):
                prev = mask_sb[0:1, i, :] if j == 0 else acc_sb[-1][:]
                a = smalls.tile([1, CHUNK], F32, tag=f"fold{j}")
                nc.vector.tensor_tensor(
                    out=a[:], in0=s_ps[32 * j:32 * j + 1, :], in1=prev,
                    op=mybir.AluOpType.add,
                )
                acc_sb.append(a)
            sc_sb = acc_sb[-1]
            negm_sb = smalls.tile([1, 1], F32, tag="negm")
            nc.vector.tensor_reduce(
                out=negm_sb[:], in_=sc_sb[:],
                axis=mybir.AxisListType.X, op=mybir.AluOpType.max, negate=True,
            )
            p_sb = smalls.tile([1, CHUNK], F16, tag="p")
            z_sb = smalls.tile([1, 1], F32, tag="z")
            nc.scalar.activation(
                out=p_sb[:], in_=sc_sb[:],
                func=mybir.ActivationFunctionType.Exp,
                bias=negm_sb[0:1, :], scale=1.0, accum_out=z_sb[:],
            )
            nc.vector.tensor_copy(out=mz_all[0:1, i, 0:1], in_=negm_sb[:])
            nc.vector.tensor_copy(out=mz_all[0:1, i, 1:2], in_=z_sb[:])
            xn_sb = xnp.tile([128, SB, H2], F16, tag="xn")
            return [i, p_sb, xn_sb]

        def emit_pt(p):
            # p row -> column layout [128, SB] via PE transpose.  Deferred to
            # the NEXT unit's PE stream (after its e-groups) so the transpose
            # never waits on the softmax chain.
            i, p_sb, xn_sb = p
            pt_sb = smalls.tile([128, SB], F16, tag="pt")
            for sb in range(SB):
                t_ps = sps.tile([128, 1], F16, tag="tp", bufs=2)
                nc.tensor.transpose(
                    t_ps[:], p_sb[0:1, sb * 128:(sb + 1) * 128], ident_sb[:])
                nc.vector.tensor_copy(out=pt_sb[:, sb:sb + 1], in_=t_ps[:])
            p[1] = pt_sb

        pending = []
        for i in range(nchunk):
            if i == 0:
                xt_sb = xt0_sb
            else:
                xt_sb = xtp.tile([128, FB, CHUNK], F16, tag="xt")
                nc.sync.dma_start(out=xt_sb[:], in_=xt_ext[i])
            if pending:
                emit_xn_dma(pending[-1])  # queued behind this unit's xt

            t_sb = tp.tile([128, OB, CHUNK], F16, tag="t")
            for ob in range(OB):
                emit_egroup(i, xt_sb, t_sb, ob)

            for p in pending:
                emit_pt(p)
            s_ps = emit_scores(i, t_sb)
            while pending:
                emit_ctx(pending.pop(0))
            pending.append(emit_softmax(i, s_ps))

        if pending:
            emit_xn_dma(pending[-1])
        for p in pending:
            emit_pt(p)
        while pending:
            emit_ctx(pending.pop(0))
        nc.sync.dma_start(out=mz_out[:], in_=mz_all[0:1, :, :])

    nc.compile()
    return nc


def kernel(encoder_out, hidden, W_attn, b_attn, w_v, b_v, lengths):
    encoder_out = np.asarray(encoder_out)
    hidden = np.asarray(hidden)
    W_attn = np.asarray(W_attn)
    b_attn = np.asarray(b_attn)
    w_v = np.asarray(w_v)
    b_v = np.asarray(b_v)
    lengths = np.asarray(lengths)

    # ---- host-side work-unit schedule from the runtime lengths ----
    units = []  # (batch, s0, valid)
    for b in range(B):
        L = int(lengths[b])
        for s0 in range(0, L, CHUNK):
            units.append((b, s0, min(CHUNK, L - s0)))
    nchunk = max(1, (len(units) + N_CORES - 1) // N_CORES)

    # ---- replicated weight layouts (fp16), o-block-major ----
    # wet[ob, p, fb, q] = W_e^T[fb*128+p, ob*128+q] = W_attn[ob*128+q, H + fb*128+p]
    wet = np.ascontiguousarray(
        W_attn[:, H:].T.reshape(FB, 128, OB, 128).transpose(2, 1, 0, 3)
    ).astype(np.float16)
    # wht[ob, p, jh, q]: blocks 0..HB-1 of W_h^T; an extra block whose row
    # p=0 carries b_attn is appended only when b_attn is nonzero
    nhb = HB + 1 if np.any(b_attn) else HB
    wht_aug = np.zeros((nhb * 128, H2), np.float32)
    wht_aug[:H] = W_attn[:, :H].T
    if nhb > HB:
        wht_aug[H] = b_attn
    wht = np.ascontiguousarray(
        wht_aug.reshape(nhb, 128, OB, 128).transpose(2, 1, 0, 3)
    ).astype(np.float16)
    wv = np.ascontiguousarray(w_v[0].reshape(OB, 128).T).astype(np.float16)

    # ---- per-core gathered inputs ----
    in_maps = []
    slot_of = []  # per real unit: (core, slot)
    x16 = encoder_out.astype(np.float16)
    for c in range(N_CORES):
        cu = units[c * nchunk:(c + 1) * nchunk]
        xt = np.zeros((nchunk, 128, FB, CHUNK), np.float16)
        xn = np.zeros((nchunk, 128, CHUNK // 128, H2), np.float16)
        mask = np.full((nchunk, CHUNK), NEG + float(b_v[0]), np.float32)
        hu = np.zeros((128, nhb, nchunk), np.float16)
        if nhb > HB:
            hu[0, HB, :] = 1.0
        for slot, (b, s0, v) in enumerate(cu):
            chunk = x16[b, s0:s0 + v, :]                      # [v, 2048]
            xt[slot, :, :, :v] = chunk.T.reshape(FB, 128, v).transpose(1, 0, 2)
            # xn[slot, p, sb, d] = chunk[sb*128 + p, d]
            full = np.zeros((CHUNK, H2), np.float16)
            full[:v] = chunk
            xn[slot] = full.reshape(CHUNK // 128, 128, H2).transpose(1, 0, 2)
            mask[slot, :v] = float(b_v[0])
            hu[:, :HB, slot] = hidden[:, b].reshape(HB, 128).T
            slot_of.append((c, slot))
        in_maps.append(dict(
            xt=xt, xn=xn, mask=mask, hu=hu,
            wet=wet, wht=wht, wv=wv,
        ))

    nc = build_program(nchunk, nhb)

    def run_once():
        res = run_bass_kernel_spmd(nc, in_maps, core_ids=list(range(N_CORES)))
        negm = np.stack([res.results[c]["out_mz"][:, 0] for c in range(N_CORES)])
        zz = np.stack([res.results[c]["out_mz"][:, 1] for c in range(N_CORES)])
        ctx = np.stack([res.results[c]["out_ctx"] for c in range(N_CORES)])
        return negm, zz, ctx

    def merge(parts):
        negm, zz, ctx = parts
        # ---- exact flash-softmax merge on host ----
        out = np.zeros((B, H2), np.float32)
        ok = np.isfinite(negm).all() and np.isfinite(zz).all() and np.isfinite(ctx).all()
        for b in range(B):
            idxs = [slot_of[k] for k, (ub, _, _) in enumerate(units) if ub == b]
            ms = np.array([-float(negm[c, s]) for c, s in idxs])
            m = ms.max()
            w = np.exp(ms - m)
            Z = float(sum(wi * float(zz[c, s]) for wi, (c, s) in zip(w, idxs)))
            if not (Z > 0):
                ok = False
                Z = 1.0
            acc = np.zeros(H2, np.float64)
            for wi, (c, s) in zip(w, idxs):
                acc += wi * ctx[c, s].astype(np.float64)
            out[b] = (acc / Z).astype(np.float32)
        # context rows are convex combinations of encoder_out rows
        ok = ok and np.isfinite(out).all() and np.abs(out).max() < 50.0
        return out, ok

    out, ok = merge(run_once())
    if not ok:  # one retry on gross corruption
        out, ok = merge(run_once())
    return out



# revision 3
# speedup vs baseline: 1.7256x; 1.7256x over previous
"""Bass/Trainium2 kernel for nn_Attention (ragged masked-softmax attention).

Math (per batch b with valid length L):
    c_b      = W_h @ hidden[:, b] + b_attn                  # [2H], W_h = W_attn[:, :H]
    e[s, :]  = tanh(W_e @ x_s + c_b)                        # W_e = W_attn[:, H:]
    score[s] = w_v . e[s, :] + b_v            (s < L)
    energy   = softmax(score[:L]);  context = energy @ X[:L]

Device strategy: the ragged work is split into fixed 256-position chunks
("units", 72 total for the graded lengths), distributed evenly over 8 cores
(one identical static SPMD program).  Each unit produces flash-softmax
partials (m, Z, ctx) which the host merges exactly.

The dominant e-matmul runs in fp8e4m3 with DoubleRow perf mode (2x the fp16
PE rate).  The fp8 quantization noise n on z = W_e x feeds the scores as
wv.(tanh'(z+c) (.) n); it is suppressed by an exact rank-1 correction:
    score = wv.t~  -  a*(wv.z~ - u.x),   u = W_e^T wv  (host, exact)
with a ~= E[tanh'] = 0.6.  wv.z~ is a second scores-matmul over the raw
PSUM copy; u.x ships from the host folded into the mask row.  Measured
rel-err ~1.0e-2 (gate 2e-2).  The per-batch bias c and u.x are host-side.
"""

import numpy as np
import ml_dtypes

import concourse.bass as bass
import concourse.mybir as mybir
import concourse.tile as tile
from concourse import bacc
from concourse.bass_utils import run_bass_kernel_spmd

B, S, H = 16, 2048, 1024
H2 = 2 * H            # 2048 output features / encoder dim
CHUNK = 256           # sequence positions per work unit
N_CORES = 8
FB = H2 // 128        # 16 f-blocks of the contraction dim (encoder features)
OB = H2 // 128        # 16 o-blocks of the output features
NEG = -30000.0        # masked-score offset (exp underflows to exactly 0)
ALPHA = 0.6           # tanh'-projection coefficient of the fp8 correction
SW = 256.0            # fp8 scale on W_e
SX = 16.0             # fp8 scale on X
INV_SWSX = 1.0 / (SW * SX)

F8 = mybir.dt.float8e4
F16 = mybir.dt.float16
F32 = mybir.dt.float32
NP8 = ml_dtypes.float8_e4m3


def build_program(nchunk: int):
    nc = bacc.Bacc()

    xt_ext = nc.declare_dram_parameter("xt", [nchunk, 128, FB, CHUNK], F8, isOutput=False)
    xn_ext = nc.declare_dram_parameter("xn", [nchunk, 128, CHUNK // 128, H2], F16, isOutput=False)
    mask_ext = nc.declare_dram_parameter("mask", [nchunk, CHUNK], F32, isOutput=False)
    c_ext = nc.declare_dram_parameter("cb", [128, OB, nchunk], F32, isOutput=False)
    wet_ext = nc.declare_dram_parameter("wet", [OB, 128, FB, 128], F8, isOutput=False)
    wv_ext = nc.declare_dram_parameter("wv", [128, 2, OB], F16, isOutput=False)
    ctx_out = nc.declare_dram_parameter("out_ctx", [nchunk, H2], F32, isOutput=True)
    mz_out = nc.declare_dram_parameter("out_mz", [nchunk, 2], F32, isOutput=True)

    SB = CHUNK // 128   # s-blocks per unit for the context matmul
    DQ = H2 // 512      # 512-wide output quarters for the context matmul

    from contextlib import ExitStack
    with tile.TileContext(nc) as tc, ExitStack() as stk:
        singles = stk.enter_context(tc.tile_pool(name="singles", bufs=1))
        xtp = stk.enter_context(tc.tile_pool(name="xtp", bufs=2))
        xnp = stk.enter_context(tc.tile_pool(name="xnp", bufs=3))
        tp = stk.enter_context(tc.tile_pool(name="tp", bufs=2))
        smalls = stk.enter_context(tc.tile_pool(name="smalls", bufs=3))
        eps = stk.enter_context(tc.tile_pool(name="eps", bufs=3, space="PSUM"))
        sps = stk.enter_context(tc.tile_pool(name="sps", bufs=2, space="PSUM"))
        cps = stk.enter_context(tc.tile_pool(name="cps", bufs=2, space="PSUM"))

        # resident weights as one tile per o-block (fine-grained DMA deps so
        # the PE can start as soon as the first o-block's weights land)
        wet_sb = []
        wv_sb = singles.tile([128, 2, OB], F16)
        c_sb = singles.tile([128, OB, nchunk], F32)
        mask_sb = singles.tile([1, nchunk, CHUNK], F32)
        xt0_sb = xtp.tile([128, FB, CHUNK], F8, tag="xt")
        for ob in range(OB):
            if ob == 0:
                nc.sync.dma_start(out=xt0_sb[:], in_=xt_ext[0])
            w1 = singles.tile([128, FB, 128], F8, tag=f"wet{ob}")
            nc.sync.dma_start(out=w1[:], in_=wet_ext[ob])
            wet_sb.append(w1)
            if ob == 0:
                nc.sync.dma_start(out=c_sb[:], in_=c_ext[:])
                nc.sync.dma_start(out=wv_sb[:], in_=wv_ext[:])
                nc.sync.dma_start(out=mask_sb[0:1, :, :], in_=mask_ext[:])
        mz_all = singles.tile([1, nchunk, 2], F32)
        ident_sb = singles.tile([1, 1], F16)
        nc.vector.memset(ident_sb[:], 1.0)

        def emit_xn_dma(p):
            i, xn_sb = p[0], p[2]
            nc.sync.dma_start(out=xn_sb[:], in_=xn_ext[i])

        def emit_ctx(p):
            i, pt_sb, xn_sb = p[0], p[1], p[2]
            ctx_sb = smalls.tile([1, H2], F32, tag="ctx")
            for dq in range(DQ):
                ctx_ps = cps.tile([1, 512], F32, tag="cps")
                for sb in range(SB):
                    nc.tensor.matmul(
                        ctx_ps[:],
                        lhsT=pt_sb[:, sb:sb + 1],
                        rhs=xn_sb[:, sb, dq * 512:(dq + 1) * 512],
                        start=(sb == 0), stop=(sb == SB - 1),
                    )
                if dq % 2 == 0:
                    nc.vector.tensor_copy(out=ctx_sb[0:1, dq * 512:(dq + 1) * 512], in_=ctx_ps[:])
                else:
                    nc.scalar.copy(out=ctx_sb[0:1, dq * 512:(dq + 1) * 512], in_=ctx_ps[:])
            nc.sync.dma_start(out=ctx_out[i], in_=ctx_sb[0:1, :])

        def emit_egroup(i, xt_sb, t_sb, d_sb, ob):
            e_ps = eps.tile([128, CHUNK], F32, tag="e")
            for fb in range(0, FB, 2):
                nc.tensor.matmul(
                    e_ps[:],
                    lhsT=wet_sb[ob][:, fb:fb + 2, :],
                    rhs=xt_sb[:, fb:fb + 2, :],
                    start=(fb == 0), stop=(fb == FB - 2),
                    perf_mode=mybir.MatmulPerfMode.DoubleRow,
                )
            # raw-z copy for the correction scores (scaled back to z units)
            nc.vector.tensor_scalar_mul(d_sb[:, ob, :], e_ps[:], INV_SWSX)
            nc.scalar.activation(
                out=t_sb[:, ob, :], in_=e_ps[:],
                func=mybir.ActivationFunctionType.Tanh,
                bias=c_sb[:, ob, i:i + 1], scale=INV_SWSX,
            )

        def emit_scores(i, t_sb, d_sb):
            # scores[s] = sum_o wv[o] t[o, s] - a * sum_o wv[o] z~[o, s]
            # as 8 partial rows: 4 (t) + 4 (z) on the 4 PE column groups
            s_ps = sps.tile([128, CHUNK], F32, tag="st", bufs=1)
            z_ps = sps.tile([128, CHUNK], F32, tag="sz", bufs=1)
            for r in range(OB // 4):
                for j in range(4):
                    ob = r * 4 + j
                    nc.tensor.matmul(
                        s_ps[32 * j:32 * j + 1, :],
                        lhsT=wv_sb[:, 0, ob:ob + 1],
                        rhs=t_sb[:, ob, :],
                        start=(r == 0), stop=(r == OB // 4 - 1),
                        tile_position=(0, 32 * j),
                    )
                    nc.tensor.matmul(
                        z_ps[32 * j:32 * j + 1, :],
                        lhsT=wv_sb[:, 1, ob:ob + 1],
                        rhs=d_sb[:, ob, :],
                        start=(r == 0), stop=(r == OB // 4 - 1),
                        tile_position=(0, 32 * j),
                    )
            return s_ps, z_ps

        def emit_softmax(i, s_ps, z_ps):
            # masked softmax partials: fold the 8 partial rows + mask/base row
            # (DVE may read at most one PSUM operand per op -> serial chain)
            acc_sb = []
            for j in range(8):
                src = (s_ps if j < 4 else z_ps)[32 * (j % 4):32 * (j % 4) + 1, :]
                prev = mask_sb[0:1, i, :] if j == 0 else acc_sb[-1][:]
                a = smalls.tile([1, CHUNK], F32, tag=f"fold{j}")
                nc.vector.tensor_tensor(
                    out=a[:], in0=src, in1=prev,
                    op=mybir.AluOpType.add,
                )
                acc_sb.append(a)
            sc_sb = acc_sb[-1]
            negm_sb = smalls.tile([1, 1], F32, tag="negm")
            nc.vector.tensor_reduce(
                out=negm_sb[:], in_=sc_sb[:],
                axis=mybir.AxisListType.X, op=mybir.AluOpType.max, negate=True,
            )
            p_sb = smalls.tile([1, CHUNK], F16, tag="p")
            z_sb = smalls.tile([1, 1], F32, tag="z")
            nc.scalar.activation(
                out=p_sb[:], in_=sc_sb[:],
                func=mybir.ActivationFunctionType.Exp,
                bias=negm_sb[0:1, :], scale=1.0, accum_out=z_sb[:],
            )
            nc.vector.tensor_copy(out=mz_all[0:1, i, 0:1], in_=negm_sb[:])
            nc.vector.tensor_copy(out=mz_all[0:1, i, 1:2], in_=z_sb[:])
            xn_sb = xnp.tile([128, SB, H2], F16, tag="xn")
            return [i, p_sb, xn_sb]

        def emit_pt(p):
            # p row -> column layout [128, SB] via PE transpose.  Deferred to
            # the NEXT unit's PE stream (after its e-groups) so the transpose
            # never waits on the softmax chain.
            i, p_sb, xn_sb = p
            pt_sb = smalls.tile([128, SB], F16, tag="pt")
            for sb in range(SB):
                t_ps = sps.tile([128, 1], F16, tag="tp", bufs=1)
                nc.tensor.transpose(
                    t_ps[:], p_sb[0:1, sb * 128:(sb + 1) * 128], ident_sb[:])
                nc.vector.tensor_copy(out=pt_sb[:, sb:sb + 1], in_=t_ps[:])
            p[1] = pt_sb

        pending = []
        for i in range(nchunk):
            if i == 0:
                xt_sb = xt0_sb
            else:
                xt_sb = xtp.tile([128, FB, CHUNK], F8, tag="xt")
                nc.sync.dma_start(out=xt_sb[:], in_=xt_ext[i])
            if pending:
                emit_xn_dma(pending[-1])  # queued behind this unit's xt

            t_sb = tp.tile([128, OB, CHUNK], F16, tag="t")
            d_sb = tp.tile([128, OB, CHUNK], F16, tag="d")
            for ob in range(OB):
                emit_egroup(i, xt_sb, t_sb, d_sb, ob)

            for p in pending:
                emit_pt(p)
            s_ps, z_ps = emit_scores(i, t_sb, d_sb)
            while pending:
                emit_ctx(pending.pop(0))
            pending.append(emit_softmax(i, s_ps, z_ps))

        if pending:
            emit_xn_dma(pending[-1])
        for p in pending:
            emit_pt(p)
        while pending:
            emit_ctx(pending.pop(0))
        nc.sync.dma_start(out=mz_out[:], in_=mz_all[0:1, :, :])

    nc.compile()
    return nc


def kernel(encoder_out, hidden, W_attn, b_attn, w_v, b_v, lengths):
    encoder_out = np.asarray(encoder_out)
    hidden = np.asarray(hidden)
    W_attn = np.asarray(W_attn)
    b_attn = np.asarray(b_attn)
    w_v = np.asarray(w_v)
    b_v = np.asarray(b_v)
    lengths = np.asarray(lengths)

    # ---- host-side work-unit schedule from the runtime lengths ----
    units = []  # (batch, s0, valid)
    for b in range(B):
        L = int(lengths[b])
        for s0 in range(0, L, CHUNK):
            units.append((b, s0, min(CHUNK, L - s0)))
    nchunk = max(1, (len(units) + N_CORES - 1) // N_CORES)

    W_e = W_attn[:, H:]                                    # [2H, 2H]
    # exact host-side per-batch bias and rank-1 score linearization
    C = hidden.T @ W_attn[:, :H].T + b_attn                # [B, 2H]
    u = W_e.T @ w_v[0]                                     # [2H]
    lin = encoder_out.reshape(-1, H2) @ u                  # [B*S]
    lin = lin.reshape(B, S)

    # ---- replicated weight layouts (fp8 DoubleRow), o-block-major ----
    # wet[ob, p, fb, q] = W_e^T[fb*128+p, ob*128+q] * SW
    wet = np.ascontiguousarray(
        W_e.T.reshape(FB, 128, OB, 128).transpose(2, 1, 0, 3) * SW
    ).astype(NP8)
    # wv plane 0: t-scores weights; plane 1: -alpha * wv for the z-correction
    wv2 = np.stack([w_v[0].reshape(OB, 128).T,
                    (-ALPHA) * w_v[0].reshape(OB, 128).T], axis=1)
    wv2 = np.ascontiguousarray(wv2).astype(np.float16)

    # ---- per-core gathered inputs ----
    in_maps = []
    slot_of = []  # per real unit: (core, slot)
    x16 = encoder_out.astype(np.float16)
    for c in range(N_CORES):
        cu = units[c * nchunk:(c + 1) * nchunk]
        xt = np.zeros((nchunk, 128, FB, CHUNK), NP8)
        xn = np.zeros((nchunk, 128, CHUNK // 128, H2), np.float16)
        mask = np.full((nchunk, CHUNK), NEG + float(b_v[0]), np.float32)
        cb = np.zeros((128, OB, nchunk), np.float32)
        for slot, (b, s0, v) in enumerate(cu):
            chunk = encoder_out[b, s0:s0 + v, :]                 # [v, 2048]
            xt[slot, :, :, :v] = (
                (chunk.T * SX).reshape(FB, 128, v).transpose(1, 0, 2).astype(NP8))
            # xn[slot, p, sb, d] = chunk[sb*128 + p, d]
            full = np.zeros((CHUNK, H2), np.float16)
            full[:v] = x16[b, s0:s0 + v, :]
            xn[slot] = full.reshape(CHUNK // 128, 128, H2).transpose(1, 0, 2)
            mask[slot, :v] = ALPHA * lin[b, s0:s0 + v] + float(b_v[0])
            cb[:, :, slot] = C[b].reshape(OB, 128).T
            slot_of.append((c, slot))
        in_maps.append(dict(
            xt=xt, xn=xn, mask=mask, cb=cb,
            wet=wet, wv=wv2,
        ))

    nc = build_program(nchunk)

    def run_once():
        res = run_bass_kernel_spmd(nc, in_maps, core_ids=list(range(N_CORES)))
        negm = np.stack([res.results[c]["out_mz"][:, 0] for c in range(N_CORES)])
        zz = np.stack([res.results[c]["out_mz"][:, 1] for c in range(N_CORES)])
        ctx = np.stack([res.results[c]["out_ctx"] for c in range(N_CORES)])
        return negm, zz, ctx

    def merge(parts):
        negm, zz, ctx = parts
        # ---- exact flash-softmax merge on host ----
        out = np.zeros((B, H2), np.float32)
        ok = np.isfinite(negm).all() and np.isfinite(zz).all() and np.isfinite(ctx).all()
        for b in range(B):
            idxs = [slot_of[k] for k, (ub, _, _) in enumerate(units) if ub == b]
            ms = np.array([-float(negm[c, s]) for c, s in idxs])
            m = ms.max()
            w = np.exp(ms - m)
            Z = float(sum(wi * float(zz[c, s]) for wi, (c, s) in zip(w, idxs)))
            if not (Z > 0):
                ok = False
                Z = 1.0
            acc = np.zeros(H2, np.float64)
            for wi, (c, s) in zip(w, idxs):
                acc += wi * ctx[c, s].astype(np.float64)
            out[b] = (acc / Z).astype(np.float32)
        # context rows are convex combinations of encoder_out rows
        ok = ok and np.isfinite(out).all() and np.abs(out).max() < 50.0
        return out, ok

    out, ok = merge(run_once())
    if not ok:  # one retry on gross corruption
        out, ok = merge(run_once())
    return out


# revision 11
# speedup vs baseline: 1.7292x; 1.0021x over previous
"""Bass/Trainium2 kernel for nn_Attention (ragged masked-softmax attention).

Math (per batch b with valid length L):
    c_b      = W_h @ hidden[:, b] + b_attn                  # [2H], W_h = W_attn[:, :H]
    e[s, :]  = tanh(W_e @ x_s + c_b)                        # W_e = W_attn[:, H:]
    score[s] = w_v . e[s, :] + b_v            (s < L)
    energy   = softmax(score[:L]);  context = energy @ X[:L]

Device strategy: the ragged work is split into fixed 256-position chunks
("units", 72 total for the graded lengths), distributed evenly over 8 cores
(one identical static SPMD program).  Each unit produces flash-softmax
partials (m, Z, ctx) which the host merges exactly.

The dominant e-matmul runs in fp8e4m3 with DoubleRow perf mode (2x the fp16
PE rate).  The fp8 quantization noise n on z = W_e x feeds the scores as
wv.(tanh'(z+c) (.) n); it is suppressed by an exact rank-1 correction:
    score = wv.t~  -  a*(wv.z~ - u.x),   u = W_e^T wv  (host, exact)
with a ~= E[tanh'] = 0.6.  wv.z~ is a second scores-matmul over the raw
PSUM copy; u.x ships from the host folded into the mask row.  Measured
rel-err ~1.0e-2 (gate 2e-2).  The per-batch bias c and u.x are host-side.
"""

import numpy as np
import ml_dtypes

import concourse.bass as bass
import concourse.mybir as mybir
import concourse.tile as tile
from concourse import bacc
from concourse.bass_utils import run_bass_kernel_spmd

B, S, H = 16, 2048, 1024
H2 = 2 * H            # 2048 output features / encoder dim
CHUNK = 256           # sequence positions per work unit
N_CORES = 8
FB = H2 // 128        # 16 f-blocks of the contraction dim (encoder features)
OB = H2 // 128        # 16 o-blocks of the output features
NEG = -30000.0        # masked-score offset (exp underflows to exactly 0)
ALPHA = 0.6           # tanh'-projection coefficient of the fp8 correction
SW = 256.0            # fp8 scale on W_e
SX = 16.0             # fp8 scale on X
INV_SWSX = 1.0 / (SW * SX)

F8 = mybir.dt.float8e4
F16 = mybir.dt.float16
F32 = mybir.dt.float32
NP8 = ml_dtypes.float8_e4m3


def build_program(nchunk: int):
    nc = bacc.Bacc()

    xt_ext = nc.declare_dram_parameter("xt", [nchunk, 128, FB, CHUNK], F8, isOutput=False)
    xn_ext = nc.declare_dram_parameter("xn", [nchunk, 128, CHUNK // 128, H2], F16, isOutput=False)
    mask_ext = nc.declare_dram_parameter("mask", [nchunk, CHUNK], F32, isOutput=False)
    c_ext = nc.declare_dram_parameter("cb", [128, OB, nchunk], F32, isOutput=False)
    wet_ext = nc.declare_dram_parameter("wet", [OB, 128, FB, 128], F8, isOutput=False)
    wv_ext = nc.declare_dram_parameter("wv", [128, OB], F16, isOutput=False)
    ctx_out = nc.declare_dram_parameter("out_ctx", [nchunk, H2], F32, isOutput=True)
    mz_out = nc.declare_dram_parameter("out_mz", [nchunk, 2], F32, isOutput=True)

    SB = CHUNK // 128   # s-blocks per unit for the context matmul
    DQ = H2 // 512      # 512-wide output quarters for the context matmul

    from contextlib import ExitStack
    with tile.TileContext(nc) as tc, ExitStack() as stk:
        singles = stk.enter_context(tc.tile_pool(name="singles", bufs=1))
        xtp = stk.enter_context(tc.tile_pool(name="xtp", bufs=2))
        xnp = stk.enter_context(tc.tile_pool(name="xnp", bufs=3))
        tp = stk.enter_context(tc.tile_pool(name="tp", bufs=2))
        smalls = stk.enter_context(tc.tile_pool(name="smalls", bufs=3))
        eps = stk.enter_context(tc.tile_pool(name="eps", bufs=3, space="PSUM"))
        sps = stk.enter_context(tc.tile_pool(name="sps", bufs=2, space="PSUM"))
        cps = stk.enter_context(tc.tile_pool(name="cps", bufs=2, space="PSUM"))

        # resident weights as one tile per o-block (fine-grained DMA deps so
        # the PE can start as soon as the first o-block's weights land)
        wet_sb = []
        wv_sb = singles.tile([128, OB], F16)
        c_sb = singles.tile([128, OB, nchunk], F32)
        mask_sb = singles.tile([1, nchunk, CHUNK], F32)
        xt0_sb = xtp.tile([128, FB, CHUNK], F8, tag="xt")
        for ob in range(OB):
            if ob == 0:
                nc.sync.dma_start(out=xt0_sb[:], in_=xt_ext[0])
            w1 = singles.tile([128, FB, 128], F8, tag=f"wet{ob}")
            nc.sync.dma_start(out=w1[:], in_=wet_ext[ob])
            wet_sb.append(w1)
            if ob == 0:
                nc.sync.dma_start(out=c_sb[:], in_=c_ext[:])
                nc.sync.dma_start(out=wv_sb[:], in_=wv_ext[:])
                nc.sync.dma_start(out=mask_sb[0:1, :, :], in_=mask_ext[:])
        mz_all = singles.tile([1, nchunk, 2], F32)
        ident_sb = singles.tile([1, 1], F16)
        nc.vector.memset(ident_sb[:], 1.0)

        def emit_xn_dma(p):
            i, xn_sb = p[0], p[2]
            nc.sync.dma_start(out=xn_sb[:], in_=xn_ext[i])

        def emit_ctx(p):
            # 4 output quarters on the 4 PE column groups, running concurrently
            i, pt_sb, xn_sb = p[0], p[1], p[2]
            ctx_sb = smalls.tile([1, H2], F32, tag="ctx")
            ctx_ps = cps.tile([128, 512], F32, tag="cps")
            for dq in range(DQ):
                for sb in range(SB):
                    nc.tensor.matmul(
                        ctx_ps[32 * dq:32 * dq + 1, :],
                        lhsT=pt_sb[:, sb:sb + 1],
                        rhs=xn_sb[:, sb, dq * 512:(dq + 1) * 512],
                        start=(sb == 0), stop=(sb == SB - 1),
                        tile_position=(0, 32 * dq),
                    )
            for dq in range(DQ):
                if dq % 2 == 0:
                    nc.vector.tensor_copy(
                        out=ctx_sb[0:1, dq * 512:(dq + 1) * 512],
                        in_=ctx_ps[32 * dq:32 * dq + 1, :])
                else:
                    nc.scalar.copy(
                        out=ctx_sb[0:1, dq * 512:(dq + 1) * 512],
                        in_=ctx_ps[32 * dq:32 * dq + 1, :])
            nc.sync.dma_start(out=ctx_out[i], in_=ctx_sb[0:1, :])

        def emit_egroup(i, xt_sb, td_sb, ob):
            e_ps = eps.tile([128, CHUNK], F32, tag="e")
            for fb in range(0, FB, 2):
                nc.tensor.matmul(
                    e_ps[:],
                    lhsT=wet_sb[ob][:, fb:fb + 2, :],
                    rhs=xt_sb[:, fb:fb + 2, :],
                    start=(fb == 0), stop=(fb == FB - 2),
                    perf_mode=mybir.MatmulPerfMode.DoubleRow,
                )
            # raw-z copy, pre-scaled by -a/(SW*SX) so the correction rides the
            # same wv matmul as the tanh half (score row = wv.t + wv.d)
            nc.vector.tensor_scalar_mul(td_sb[:, ob, 1, :], e_ps[:], -ALPHA * INV_SWSX)
            nc.scalar.activation(
                out=td_sb[:, ob, 0, :], in_=e_ps[:],
                func=mybir.ActivationFunctionType.Tanh,
                bias=c_sb[:, ob, i:i + 1], scale=INV_SWSX,
            )

        def emit_scores(i, td_sb):
            # score row j = sum_{ob in group j} wv[ob] . (t[ob] || d[ob]):
            # one 512-wide matmul per ob, 4 chains on the 4 PE column groups
            s_ps = sps.tile([128, 2 * CHUNK], F32, tag="st", bufs=1)
            for r in range(OB // 4):
                for j in range(4):
                    ob = r * 4 + j
                    nc.tensor.matmul(
                        s_ps[32 * j:32 * j + 1, :],
                        lhsT=wv_sb[:, ob:ob + 1],
                        rhs=td_sb[:, ob, :, :],
                        start=(r == 0), stop=(r == OB // 4 - 1),
                        tile_position=(0, 32 * j),
                    )
            return s_ps

        def emit_softmax(i, s_ps):
            # masked softmax partials: fold 4 rows x 2 halves + mask/base row
            # (DVE may read at most one PSUM operand per op -> serial chain)
            acc_sb = []
            for j in range(8):
                src = s_ps[32 * (j // 2):32 * (j // 2) + 1,
                           (j % 2) * CHUNK:(j % 2 + 1) * CHUNK]
                prev = mask_sb[0:1, i, :] if j == 0 else acc_sb[-1][:]
                a = smalls.tile([1, CHUNK], F32, tag=f"fold{j}")
                nc.vector.tensor_tensor(
                    out=a[:], in0=src, in1=prev,
                    op=mybir.AluOpType.add,
                )
                acc_sb.append(a)
            sc_sb = acc_sb[-1]
            negm_sb = smalls.tile([1, 1], F32, tag="negm")
            nc.vector.tensor_reduce(
                out=negm_sb[:], in_=sc_sb[:],
                axis=mybir.AxisListType.X, op=mybir.AluOpType.max, negate=True,
            )
            p_sb = smalls.tile([1, CHUNK], F16, tag="p")
            z_sb = smalls.tile([1, 1], F32, tag="z")
            nc.scalar.activation(
                out=p_sb[:], in_=sc_sb[:],
                func=mybir.ActivationFunctionType.Exp,
                bias=negm_sb[0:1, :], scale=1.0, accum_out=z_sb[:],
            )
            nc.vector.tensor_copy(out=mz_all[0:1, i, 0:1], in_=negm_sb[:])
            nc.vector.tensor_copy(out=mz_all[0:1, i, 1:2], in_=z_sb[:])
            xn_sb = xnp.tile([128, SB, H2], F16, tag="xn")
            return [i, p_sb, xn_sb]

        def emit_pt(p):
            # p row -> column layout [128, SB] via PE transpose.  Deferred to
            # the NEXT unit's PE stream (after its e-groups) so the transpose
            # never waits on the softmax chain.
            i, p_sb, xn_sb = p
            pt_sb = smalls.tile([128, SB], F16, tag="pt")
            for sb in range(SB):
                t_ps = sps.tile([128, 1], F16, tag="tp", bufs=2)
                nc.tensor.transpose(
                    t_ps[:], p_sb[0:1, sb * 128:(sb + 1) * 128], ident_sb[:])
                nc.vector.tensor_copy(out=pt_sb[:, sb:sb + 1], in_=t_ps[:])
            p[1] = pt_sb

        pending = []
        for i in range(nchunk):
            if i == 0:
                xt_sb = xt0_sb
            else:
                xt_sb = xtp.tile([128, FB, CHUNK], F8, tag="xt")
                nc.sync.dma_start(out=xt_sb[:], in_=xt_ext[i])
            if pending:
                emit_xn_dma(pending[-1])  # queued behind this unit's xt

            td_sb = tp.tile([128, OB, 2, CHUNK], F16, tag="td")
            for ob in range(OB):
                emit_egroup(i, xt_sb, td_sb, ob)

            for p in pending:
                emit_pt(p)
            s_ps = emit_scores(i, td_sb)
            while pending:
                emit_ctx(pending.pop(0))
            pending.append(emit_softmax(i, s_ps))

        if pending:
            emit_xn_dma(pending[-1])
        for p in pending:
            emit_pt(p)
        while pending:
            emit_ctx(pending.pop(0))
        nc.sync.dma_start(out=mz_out[:], in_=mz_all[0:1, :, :])

    nc.compile()
    return nc


def kernel(encoder_out, hidden, W_attn, b_attn, w_v, b_v, lengths):
    encoder_out = np.asarray(encoder_out)
    hidden = np.asarray(hidden)
    W_attn = np.asarray(W_attn)
    b_attn = np.asarray(b_attn)
    w_v = np.asarray(w_v)
    b_v = np.asarray(b_v)
    lengths = np.asarray(lengths)

    # ---- host-side work-unit schedule from the runtime lengths ----
    units = []  # (batch, s0, valid)
    for b in range(B):
        L = int(lengths[b])
        for s0 in range(0, L, CHUNK):
            units.append((b, s0, min(CHUNK, L - s0)))
    nchunk = max(1, (len(units) + N_CORES - 1) // N_CORES)

    W_e = W_attn[:, H:]                                    # [2H, 2H]
    # exact host-side per-batch bias and rank-1 score linearization
    C = hidden.T @ W_attn[:, :H].T + b_attn                # [B, 2H]
    u = W_e.T @ w_v[0]                                     # [2H]
    lin = encoder_out.reshape(-1, H2) @ u                  # [B*S]
    lin = lin.reshape(B, S)

    # ---- replicated weight layouts (fp8 DoubleRow), o-block-major ----
    # wet[ob, p, fb, q] = W_e^T[fb*128+p, ob*128+q] * SW
    wet = np.ascontiguousarray(
        W_e.T.reshape(FB, 128, OB, 128).transpose(2, 1, 0, 3) * SW
    ).astype(NP8)
    wv2 = np.ascontiguousarray(w_v[0].reshape(OB, 128).T).astype(np.float16)

    # ---- per-core gathered inputs ----
    in_maps = []
    slot_of = []  # per real unit: (core, slot)
    x16 = encoder_out.astype(np.float16)
    for c in range(N_CORES):
        cu = units[c * nchunk:(c + 1) * nchunk]
        xt = np.zeros((nchunk, 128, FB, CHUNK), NP8)
        xn = np.zeros((nchunk, 128, CHUNK // 128, H2), np.float16)
        mask = np.full((nchunk, CHUNK), NEG + float(b_v[0]), np.float32)
        cb = np.zeros((128, OB, nchunk), np.float32)
        for slot, (b, s0, v) in enumerate(cu):
            chunk = encoder_out[b, s0:s0 + v, :]                 # [v, 2048]
            xt[slot, :, :, :v] = (
                (chunk.T * SX).reshape(FB, 128, v).transpose(1, 0, 2).astype(NP8))
            # xn[slot, p, sb, d] = chunk[sb*128 + p, d]
            full = np.zeros((CHUNK, H2), np.float16)
            full[:v] = x16[b, s0:s0 + v, :]
            xn[slot] = full.reshape(CHUNK // 128, 128, H2).transpose(1, 0, 2)
            mask[slot, :v] = ALPHA * lin[b, s0:s0 + v] + float(b_v[0])
            cb[:, :, slot] = C[b].reshape(OB, 128).T
            slot_of.append((c, slot))
        in_maps.append(dict(
            xt=xt, xn=xn, mask=mask, cb=cb,
            wet=wet, wv=wv2,
        ))

    nc = build_program(nchunk)

    def run_once():
        res = run_bass_kernel_spmd(nc, in_maps, core_ids=list(range(N_CORES)))
        negm = np.stack([res.results[c]["out_mz"][:, 0] for c in range(N_CORES)])
        zz = np.stack([res.results[c]["out_mz"][:, 1] for c in range(N_CORES)])
        ctx = np.stack([res.results[c]["out_ctx"] for c in range(N_CORES)])
        return negm, zz, ctx

    def merge(parts):
        negm, zz, ctx = parts
        # ---- exact flash-softmax merge on host ----
        out = np.zeros((B, H2), np.float32)
        ok = np.isfinite(negm).all() and np.isfinite(zz).all() and np.isfinite(ctx).all()
        for b in range(B):
            idxs = [slot_of[k] for k, (ub, _, _) in enumerate(units) if ub == b]
            ms = np.array([-float(negm[c, s]) for c, s in idxs])
            m = ms.max()
            w = np.exp(ms - m)
            Z = float(sum(wi * float(zz[c, s]) for wi, (c, s) in zip(w, idxs)))
            if not (Z > 0):
                ok = False
                Z = 1.0
            acc = np.zeros(H2, np.float64)
            for wi, (c, s) in zip(w, idxs):
                acc += wi * ctx[c, s].astype(np.float64)
            out[b] = (acc / Z).astype(np.float32)
        # context rows are convex combinations of encoder_out rows
        ok = ok and np.isfinite(out).all() and np.abs(out).max() < 50.0
        return out, ok

    out, ok = merge(run_once())
    if not ok:  # one retry on gross corruption
        out, ok = merge(run_once())
    return out


# revision 29
# speedup vs baseline: 1.9872x; 1.1492x over previous
"""Bass/Trainium2 kernel for nn_Attention (ragged masked-softmax attention).

Math (per batch b with valid length L):
    c_b      = W_h @ hidden[:, b] + b_attn                  # [2H], W_h = W_attn[:, :H]
    e[s, :]  = tanh(W_e @ x_s + c_b)                        # W_e = W_attn[:, H:]
    score[s] = w_v . e[s, :] + b_v            (s < L)
    energy   = softmax(score[:L]);  context = energy @ X[:L]

Device strategy: the ragged work is split into fixed 256-position chunks
("units", 72 total for the graded lengths), distributed evenly over 8 cores
(one identical static SPMD program).  Each unit produces flash-softmax
partials (m, Z, ctx) which the host merges exactly.

The dominant e-matmul runs in fp8e4m3 with DoubleRow perf mode (2x the fp16
PE rate).  The fp8 quantization noise n on z = W_e x feeds the scores as
wv.(tanh'(z+c) (.) n); it is suppressed by an exact rank-1 correction:
    score = wv.t~  -  a*(wv.z~ - u.x),   u = W_e^T wv  (host, exact)
with a ~= E[tanh'] = 0.6.  wv.z~ is a second scores-matmul over the raw
PSUM copy; u.x ships from the host folded into the mask row.  Measured
rel-err ~1.0e-2 (gate 2e-2).  The per-batch bias c and u.x are host-side.
"""

import numpy as np
import ml_dtypes

import concourse.bass as bass
import concourse.mybir as mybir
import concourse.tile as tile
from concourse import bacc
from concourse.bass_utils import run_bass_kernel_spmd

B, S, H = 16, 2048, 1024
H2 = 2 * H            # 2048 output features / encoder dim
CHUNK = 256           # sequence positions per work unit
N_CORES = 8
FB = H2 // 128        # 16 f-blocks of the contraction dim (encoder features)
OB = H2 // 128        # 16 o-blocks of the output features
NEG = -30000.0        # masked-score offset (exp underflows to exactly 0)
ALPHA = 0.6           # tanh'-projection coefficient of the fp8 correction
SW = 256.0            # fp8 scale on W_e
SX = 16.0             # fp8 scale on X
SV = 1024.0           # fp8 scale on the correction rows v = W8^T wv
INV_SWSX = 1.0 / (SW * SX)
CH = -ALPHA / (SV * SX)   # fold scale of the correction rows

F8 = mybir.dt.float8e4
F16 = mybir.dt.float16
F32 = mybir.dt.float32
NP8 = ml_dtypes.float8_e4m3


def build_program(nchunk: int):
    nc = bacc.Bacc()

    xt_ext = nc.declare_dram_parameter("xt", [nchunk, 128, FB, CHUNK], F8, isOutput=False)
    xn_ext = nc.declare_dram_parameter("xn", [nchunk, 128, CHUNK // 128, H2], F16, isOutput=False)
    mask_ext = nc.declare_dram_parameter("mask", [nchunk, CHUNK], F32, isOutput=False)
    c_ext = nc.declare_dram_parameter("cb", [128, OB, nchunk], F32, isOutput=False)
    wet_ext = nc.declare_dram_parameter("wet", [OB, 128, FB, 128], F8, isOutput=False)
    wv_ext = nc.declare_dram_parameter("wv", [128, OB], F16, isOutput=False)
    vrow_ext = nc.declare_dram_parameter("vrow", [128, FB, 128], F8, isOutput=False)
    ctx_out = nc.declare_dram_parameter("out_ctx", [nchunk, H2], F32, isOutput=True)
    mz_out = nc.declare_dram_parameter("out_mz", [nchunk, 2], F32, isOutput=True)

    SB = CHUNK // 128   # s-blocks per unit for the context matmul
    DQ = H2 // 512      # 512-wide output quarters for the context matmul

    from contextlib import ExitStack
    with tile.TileContext(nc) as tc, ExitStack() as stk:
        singles = stk.enter_context(tc.tile_pool(name="singles", bufs=1))
        xtp = stk.enter_context(tc.tile_pool(name="xtp", bufs=2))
        xnp = stk.enter_context(tc.tile_pool(name="xnp", bufs=3))
        tp = stk.enter_context(tc.tile_pool(name="tp", bufs=2))
        smalls = stk.enter_context(tc.tile_pool(name="smalls", bufs=3))
        eps = stk.enter_context(tc.tile_pool(name="eps", bufs=3, space="PSUM"))
        sps = stk.enter_context(tc.tile_pool(name="sps", bufs=2, space="PSUM"))
        cps = stk.enter_context(tc.tile_pool(name="cps", bufs=1, space="PSUM"))

        # resident weights as one tile per o-block (fine-grained DMA deps so
        # the PE can start as soon as the first o-block's weights land)
        wet_sb = []
        wv_sb = singles.tile([128, OB], F16)
        vrow_sb = singles.tile([128, FB, 128], F8)
        c_sb = singles.tile([128, OB, nchunk], F32)
        mask_sb = singles.tile([1, nchunk, CHUNK], F32)
        xt0_sb = xtp.tile([128, FB, CHUNK], F8, tag="xt")
        for ob in range(OB):
            if ob == 0:
                nc.sync.dma_start(out=xt0_sb[:], in_=xt_ext[0])
            w1 = singles.tile([128, FB, 128], F8, tag=f"wet{ob}")
            nc.sync.dma_start(out=w1[:], in_=wet_ext[ob])
            wet_sb.append(w1)
            if ob == 0:
                nc.sync.dma_start(out=c_sb[:], in_=c_ext[:])
                nc.sync.dma_start(out=wv_sb[:], in_=wv_ext[:])
                nc.sync.dma_start(out=vrow_sb[:], in_=vrow_ext[:])
                nc.sync.dma_start(out=mask_sb[0:1, :, :], in_=mask_ext[:])
        mz_all = singles.tile([1, nchunk, 2], F32)
        ident_sb = singles.tile([1, 1], F16)
        nc.vector.memset(ident_sb[:], 1.0)

        def emit_xn_dma(p):
            i, xn_sb = p[0], p[2]
            nc.sync.dma_start(out=xn_sb[:], in_=xn_ext[i])

        def emit_ctx(p):
            # 4 output quarters on the 4 PE column groups, running concurrently
            i, pt_sb, xn_sb = p[0], p[1], p[2]
            ctx_sb = smalls.tile([1, H2], F32, tag="ctx")
            ctx_ps = cps.tile([128, 512], F32, tag="cps")
            for dq in range(DQ):
                for sb in range(SB):
                    nc.tensor.matmul(
                        ctx_ps[32 * dq:32 * dq + 1, :],
                        lhsT=pt_sb[:, sb:sb + 1],
                        rhs=xn_sb[:, sb, dq * 512:(dq + 1) * 512],
                        start=(sb == 0), stop=(sb == SB - 1),
                        tile_position=(0, 32 * dq),
                    )
            for dq in range(DQ):
                if dq % 2 == 0:
                    nc.vector.tensor_copy(
                        out=ctx_sb[0:1, dq * 512:(dq + 1) * 512],
                        in_=ctx_ps[32 * dq:32 * dq + 1, :])
                else:
                    nc.scalar.copy(
                        out=ctx_sb[0:1, dq * 512:(dq + 1) * 512],
                        in_=ctx_ps[32 * dq:32 * dq + 1, :])
            nc.sync.dma_start(out=ctx_out[i], in_=ctx_sb[0:1, :])

        def emit_egroup(i, xt_sb, t_sb, ob):
            e_ps = eps.tile([128, CHUNK], F32, tag="e")
            for fb in range(0, FB, 2):
                nc.tensor.matmul(
                    e_ps[:],
                    lhsT=wet_sb[ob][:, fb:fb + 2, :],
                    rhs=xt_sb[:, fb:fb + 2, :],
                    start=(fb == 0), stop=(fb == FB - 2),
                    perf_mode=mybir.MatmulPerfMode.DoubleRow,
                )
            nc.scalar.activation(
                out=t_sb[:, ob, :], in_=e_ps[:],
                func=mybir.ActivationFunctionType.Tanh,
                bias=c_sb[:, ob, i:i + 1], scale=INV_SWSX,
            )

        def emit_vchain(i, xt_sb):
            # correction rows hi/lo of v = W8^T wv over the same fp8 X, as two
            # single-row chains on different PE column groups (run concurrently)
            v_ps = sps.tile([128, CHUNK], F32, tag="v", bufs=1)
            for fb in range(0, FB, 2):
                nc.tensor.matmul(
                    v_ps[:],
                    lhsT=vrow_sb[:, fb:fb + 2, :],
                    rhs=xt_sb[:, fb:fb + 2, :],
                    start=(fb == 0), stop=(fb == FB - 2),
                    perf_mode=mybir.MatmulPerfMode.DoubleRow,
                )
            h_sb = smalls.tile([1, 2, CHUNK], F32, tag="h")
            nc.scalar.mul(h_sb[0:1, 0, :], v_ps[0:1, :], CH)
            nc.scalar.mul(h_sb[0:1, 1, :], v_ps[32:33, :], CH)
            return h_sb

        def emit_scores(i, t_sb):
            # scores[s] = sum_o wv[o] t[o, s] -> 4 partial rows on the 4 PE
            # column groups running concurrently
            s_ps = sps.tile([128, CHUNK], F32, tag="st", bufs=1)
            for r in range(OB // 4):
                for j in range(4):
                    ob = r * 4 + j
                    nc.tensor.matmul(
                        s_ps[32 * j:32 * j + 1, :],
                        lhsT=wv_sb[:, ob:ob + 1],
                        rhs=t_sb[:, ob, :],
                        start=(r == 0), stop=(r == OB // 4 - 1),
                        tile_position=(0, 32 * j),
                    )
            return s_ps

        def emit_softmax(i, s_ps, h_sb):
            # masked softmax partials: fold 4 t-rows + 2 correction rows + mask
            # (DVE may read at most one PSUM operand per op -> serial chain)
            acc_sb = []
            srcs = [s_ps[32 * j:32 * j + 1, :] for j in range(4)] + \
                   [h_sb[0:1, 0, :], h_sb[0:1, 1, :]]
            for j, src in enumerate(srcs):
                prev = mask_sb[0:1, i, :] if j == 0 else acc_sb[-1][:]
                a = smalls.tile([1, CHUNK], F32, tag=f"fold{j}")
                nc.vector.tensor_tensor(
                    out=a[:], in0=src, in1=prev,
                    op=mybir.AluOpType.add,
                )
                acc_sb.append(a)
            sc_sb = acc_sb[-1]
            negm_sb = smalls.tile([1, 1], F32, tag="negm")
            nc.vector.tensor_reduce(
                out=negm_sb[:], in_=sc_sb[:],
                axis=mybir.AxisListType.X, op=mybir.AluOpType.max, negate=True,
            )
            p_sb = smalls.tile([1, CHUNK], F16, tag="p")
            z_sb = smalls.tile([1, 1], F32, tag="z")
            nc.scalar.activation(
                out=p_sb[:], in_=sc_sb[:],
                func=mybir.ActivationFunctionType.Exp,
                bias=negm_sb[0:1, :], scale=1.0, accum_out=z_sb[:],
            )
            nc.vector.tensor_copy(out=mz_all[0:1, i, 0:1], in_=negm_sb[:])
            nc.vector.tensor_copy(out=mz_all[0:1, i, 1:2], in_=z_sb[:])
            xn_sb = xnp.tile([128, SB, H2], F16, tag="xn")
            return [i, p_sb, xn_sb]

        def emit_pt(p):
            # p row -> column layout [128, SB] via PE transpose.  Deferred to
            # the NEXT unit's PE stream (after its e-groups) so the transpose
            # never waits on the softmax chain.
            i, p_sb, xn_sb = p
            pt_sb = smalls.tile([128, SB], F16, tag="pt")
            for sb in range(SB):
                t_ps = sps.tile([128, 1], F16, tag="tp", bufs=2)
                nc.tensor.transpose(
                    t_ps[:], p_sb[0:1, sb * 128:(sb + 1) * 128], ident_sb[:])
                nc.vector.tensor_copy(out=pt_sb[:, sb:sb + 1], in_=t_ps[:])
            p[1] = pt_sb

        pending = []
        for i in range(nchunk):
            if i == 0:
                xt_sb = xt0_sb
            else:
                xt_sb = xtp.tile([128, FB, CHUNK], F8, tag="xt")
                nc.sync.dma_start(out=xt_sb[:], in_=xt_ext[i])
            if pending:
                emit_xn_dma(pending[-1])  # queued behind this unit's xt

            t_sb = tp.tile([128, OB, CHUNK], F16, tag="t")
            for ob in range(OB):
                emit_egroup(i, xt_sb, t_sb, ob)

            h_sb = emit_vchain(i, xt_sb)
            for p in pending:
                emit_pt(p)
            s_ps = emit_scores(i, t_sb)
            while pending:
                emit_ctx(pending.pop(0))
            pending.append(emit_softmax(i, s_ps, h_sb))

        if pending:
            emit_xn_dma(pending[-1])
        for p in pending:
            emit_pt(p)
        while pending:
            emit_ctx(pending.pop(0))
        nc.sync.dma_start(out=mz_out[:], in_=mz_all[0:1, :, :])

    nc.compile()
    return nc


def kernel(encoder_out, hidden, W_attn, b_attn, w_v, b_v, lengths):
    encoder_out = np.asarray(encoder_out)
    hidden = np.asarray(hidden)
    W_attn = np.asarray(W_attn)
    b_attn = np.asarray(b_attn)
    w_v = np.asarray(w_v)
    b_v = np.asarray(b_v)
    lengths = np.asarray(lengths)

    # ---- host-side work-unit schedule from the runtime lengths ----
    units = []  # (batch, s0, valid)
    for b in range(B):
        L = int(lengths[b])
        for s0 in range(0, L, CHUNK):
            units.append((b, s0, min(CHUNK, L - s0)))
    nchunk = max(1, (len(units) + N_CORES - 1) // N_CORES)

    W_e = W_attn[:, H:]                                    # [2H, 2H]
    # exact host-side per-batch bias and rank-1 score linearization
    C = hidden.T @ W_attn[:, :H].T + b_attn                # [B, 2H]
    u = W_e.T @ w_v[0]                                     # [2H]
    lin = encoder_out.reshape(-1, H2) @ u                  # [B*S]
    lin = lin.reshape(B, S)

    # ---- replicated weight layouts (fp8 DoubleRow), o-block-major ----
    # wet[ob, p, fb, q] = W_e^T[fb*128+p, ob*128+q] * SW
    wet = np.ascontiguousarray(
        W_e.T.reshape(FB, 128, OB, 128).transpose(2, 1, 0, 3) * SW
    ).astype(NP8)
    wv2 = np.ascontiguousarray(w_v[0].reshape(OB, 128).T).astype(np.float16)
    # correction rows from the QUANTIZED weights: v[f] = sum_o W8[o,f] wv[o]
    # wet[ob,p,fb,q] = W8^T[fb*128+p, ob*128+q]*SW
    W8T = wet.astype(np.float32).transpose(2, 1, 0, 3).reshape(H2, H2) / SW
    v = W8T @ w_v[0]                                       # [2H], exact fp32
    vh = (v * SV).astype(NP8)                              # hi row (scaled)
    vl = (v * SV - vh.astype(np.float32)).astype(NP8)      # lo residual row
    vrow = np.zeros((128, FB, 128), NP8)             # col 0 = hi, col 32 = lo
    vrow[:, :, 0] = vh.reshape(FB, 128).T
    vrow[:, :, 32] = vl.reshape(FB, 128).T

    # ---- per-core gathered inputs ----
    in_maps = []
    slot_of = []  # per real unit: (core, slot)
    x16 = encoder_out.astype(np.float16)
    for c in range(N_CORES):
        cu = units[c * nchunk:(c + 1) * nchunk]
        xt = np.zeros((nchunk, 128, FB, CHUNK), NP8)
        xn = np.zeros((nchunk, 128, CHUNK // 128, H2), np.float16)
        mask = np.full((nchunk, CHUNK), NEG + float(b_v[0]), np.float32)
        cb = np.zeros((128, OB, nchunk), np.float32)
        for slot, (b, s0, v) in enumerate(cu):
            chunk = encoder_out[b, s0:s0 + v, :]                 # [v, 2048]
            xt[slot, :, :, :v] = (
                (chunk.T * SX).reshape(FB, 128, v).transpose(1, 0, 2).astype(NP8))
            # xn[slot, p, sb, d] = chunk[sb*128 + p, d]
            full = np.zeros((CHUNK, H2), np.float16)
            full[:v] = x16[b, s0:s0 + v, :]
            xn[slot] = full.reshape(CHUNK // 128, 128, H2).transpose(1, 0, 2)
            mask[slot, :v] = ALPHA * lin[b, s0:s0 + v] + float(b_v[0])
            cb[:, :, slot] = C[b].reshape(OB, 128).T
            slot_of.append((c, slot))
        in_maps.append(dict(
            xt=xt, xn=xn, mask=mask, cb=cb,
            wet=wet, wv=wv2, vrow=vrow,
        ))

    nc = build_program(nchunk)

    def run_once():
        res = run_bass_kernel_spmd(nc, in_maps, core_ids=list(range(N_CORES)))
        negm = np.stack([res.results[c]["out_mz"][:, 0] for c in range(N_CORES)])
        zz = np.stack([res.results[c]["out_mz"][:, 1] for c in range(N_CORES)])
        ctx = np.stack([res.results[c]["out_ctx"] for c in range(N_CORES)])
        return negm, zz, ctx

    def merge(parts):
        negm, zz, ctx = parts
        # ---- exact flash-softmax merge on host ----
        out = np.zeros((B, H2), np.float32)
        ok = np.isfinite(negm).all() and np.isfinite(zz).all() and np.isfinite(ctx).all()
        for b in range(B):
            idxs = [slot_of[k] for k, (ub, _, _) in enumerate(units) if ub == b]
            ms = np.array([-float(negm[c, s]) for c, s in idxs])
            m = ms.max()
            w = np.exp(ms - m)
            Z = float(sum(wi * float(zz[c, s]) for wi, (c, s) in zip(w, idxs)))
            if not (Z > 0):
                ok = False
                Z = 1.0
            acc = np.zeros(H2, np.float64)
            for wi, (c, s) in zip(w, idxs):
                acc += wi * ctx[c, s].astype(np.float64)
            out[b] = (acc / Z).astype(np.float32)
        # context rows are convex combinations of encoder_out rows
        ok = ok and np.isfinite(out).all() and np.abs(out).max() < 50.0
        return out, ok

    out, ok = merge(run_once())
    if not ok:  # one retry on gross corruption
        out, ok = merge(run_once())
    return out
